# revision 1
# baseline (speedup 1.0000x reference)
"""Trainium2 Bass kernel: CorrelatorK3.

Math (per batch b):
    q0 = rbf_0 @ Q0_w.T + Q0_b          [N, N, F]
    q  = rbf_d @ Q_w.T  + Q_b
    r0 = rbf_0 @ R0_w.T + R0_b
    r  = rbf_d @ R_w.T  + R_b
    C[n, j] = sum_{i, f} (q0*q)[n, i, f] * (r0*r)[i, j, f] * 0.02

Sharding: data-parallel over batch B=8 across the 8 NeuronCores (one batch
per core); the output is a pure concat, no collectives.

Per-core pipeline:
  Phase 1 (stream x = flattened (u, v) index over 256*256 rows):
    - DMA natural [x, d] tiles, PE-transpose to [d, x] (fp32),
      evacuate to fp16 via ScalarE.
    - Projections as fp16 matmuls: stationary = [Q*|R*]-stacked weights
      (duplicated over both partition halves), moving = rbfT tiles.
      Row-groups 0-63 / 64-127 process the v<128 / v>=128 halves
      concurrently via tile_position.
    - Bias + elementwise products fused with PSUM evacuation
      (ScalarE Identity+bias for side-0, VectorE scalar_tensor_tensor for
      the product), written fp16 into a resident tensor R:
        R[0:64,  x] = A^T[f, x]  = (q0+b)(q+b)   (x = (n, i))
        R[64:128,x] = Bm^T[f, x] = (r0+b)(r+b)   (x = (i, j))
  Phase 2 (contraction):
    - Bm slices staged h1 -> h0 partitions by SBUF->SBUF DMA in i-blocks.
    - C[n-tile, j] += A^T[:, n-tile, i].T @ Bm^T[:, i, :] accumulated over
      all 256 i in PSUM (fp32), scaled by 0.02 on the way out.
"""

import sys

if "/opt/trn_rl_repo" not in sys.path:
    sys.path.insert(0, "/opt/trn_rl_repo")

from contextlib import ExitStack

import numpy as np

import concourse.mybir as mybir
import concourse.tile as tile
from concourse import bacc
from concourse.bass_utils import run_bass_kernel_spmd
from concourse.masks import make_identity

B, N, D, F = 8, 256, 64, 64
X = N * N  # 65536 flattened rows per batch
INTERVAL = 0.02

F32 = mybir.dt.float32
F16 = mybir.dt.float16

import os

UPB = int(os.environ.get("KERNEL_UPB", "4"))  # u-rows per phase-1 block
UB = X // (UPB * N)  # 64 phase-1 blocks
IB = 32  # i per phase-2 staging block
_PHASES = os.environ.get("KERNEL_PHASES", "12")  # debug: "1" or "2" only


def _body(ctx, tc, rbf0, rbfd, w0, wd, b0, bd, cout):
    nc = tc.nc
    Copy = mybir.ActivationFunctionType.Copy
    Ident = mybir.ActivationFunctionType.Identity
    Alu = mybir.AluOpType

    const = ctx.enter_context(tc.tile_pool(name="const", bufs=1))
    w0_sb = const.tile([128, 128], F16)
    wd_sb = const.tile([128, 128], F16)
    b0_sb = const.tile([128, 1], F32)
    bd_sb = const.tile([128, 1], F32)
    ident = const.tile([128, 128], F32)
    nc.sync.dma_start(w0_sb[:], w0[:])
    nc.sync.dma_start(wd_sb[:], wd[:])
    nc.sync.dma_start(b0_sb[:], b0[:])
    nc.sync.dma_start(bd_sb[:], bd[:])
    make_identity(nc, ident[:])

    res_pool = ctx.enter_context(tc.tile_pool(name="res", bufs=1))
    R = res_pool.tile([128, X], F16)
    R3 = R[:].rearrange("p (u v) -> p u v", v=N)  # [128, 256, 256]

    # ---------------- Phase 1: projections + products ----------------
    rbf0v = rbf0[:].rearrange("(c t p) d -> c p t d", t=2 * UPB, p=128)
    rbfdv = rbfd[:].rearrange("(c t p) d -> c p t d", t=2 * UPB, p=128)

    if "1" in _PHASES:
        _phase1(tc, rbf0v, rbfdv, w0_sb, wd_sb, b0_sb, bd_sb, ident, R3)
    if "2" in _PHASES:
        _phase2(tc, R, R3, cout)


def _phase1(tc, rbf0v, rbfdv, w0_sb, wd_sb, b0_sb, bd_sb, ident, R3):
    nc = tc.nc
    Copy = mybir.ActivationFunctionType.Copy
    Ident = mybir.ActivationFunctionType.Identity
    Alu = mybir.AluOpType
    F32R = mybir.dt.float32r
    identr = ident[:].bitcast(F32R)
    HB = UPB * 128  # half-block columns (one row-group's share)
    with (
        tc.tile_pool(name="chunk", bufs=3) as chunk_pool,
        tc.tile_pool(name="rbfT", bufs=2) as rbfT_pool,
        tc.tile_pool(name="s0p", bufs=2) as s0_pool,
        tc.tile_pool(name="pt", bufs=2, space="PSUM") as pt_pool,
        tc.tile_pool(
            name="pp", bufs=(2 if UPB <= 2 else 1), space="PSUM"
        ) as pp_pool,
    ):
        for ub in range(UB):
            ch0 = chunk_pool.tile([128, HB], F32, tag="ch0")
            chd = chunk_pool.tile([128, HB], F32, tag="chd")
            # two independent DMA queues: HWDGE (sync) + SWDGE (gpsimd)
            nc.sync.dma_start(
                ch0[:].rearrange("p (t d) -> p t d", d=D), rbf0v[ub]
            )
            nc.gpsimd.dma_start(
                chd[:].rearrange("p (t d) -> p t d", d=D), rbfdv[ub]
            )

            # transpose UPB [128, 128] sub-blocks per side; both sides share
            # one wide psum tile (each transpose stays inside one bank)
            pt = pt_pool.tile([128, 2 * HB], F32, tag="pt")
            for j in range(UPB):
                sl = slice(128 * j, 128 * (j + 1))
                sld = slice(HB + 128 * j, HB + 128 * (j + 1))
                nc.tensor.transpose(pt[:, sl], ch0[:, sl], ident[:])
                nc.tensor.transpose(pt[:, sld], chd[:, sl], ident[:])

            # evacuate both sides' transposes to fp16 in one ScalarE op
            tt = rbfT_pool.tile([128, 2 * HB], F16, tag="tt")
            nc.scalar.activation(tt[:], pt[:], Copy)

            # projections: wide psum per side, col-halves per row-group
            pp0 = pp_pool.tile([128, 2 * HB], F32, tag="pp0")
            ppd = pp_pool.tile([128, 2 * HB], F32, tag="ppd")
            nc.tensor.matmul(
                pp0[:, 0:HB], w0_sb[0:64, :], tt[0:64, 0:HB],
                start=True, stop=True, tile_position=(0, 0),
            )
            nc.tensor.matmul(
                ppd[:, 0:HB], wd_sb[0:64, :], tt[0:64, HB : 2 * HB],
                start=True, stop=True, tile_position=(0, 0),
            )
            nc.tensor.matmul(
                pp0[:, HB : 2 * HB], w0_sb[64:128, :], tt[64:128, 0:HB],
                start=True, stop=True, tile_position=(64, 0),
            )
            nc.tensor.matmul(
                ppd[:, HB : 2 * HB], wd_sb[64:128, :],
                tt[64:128, HB : 2 * HB],
                start=True, stop=True, tile_position=(64, 0),
            )

            # side-0 bias evacuation: g0 half on ScalarE, g1 half on VectorE
            s0 = s0_pool.tile([128, 2 * HB], F32, tag="s0")
            nc.scalar.activation(
                s0[:, 0:HB], pp0[:, 0:HB], Ident, bias=b0_sb[:]
            )
            nc.vector.tensor_scalar_add(
                s0[:, HB : 2 * HB], pp0[:, HB : 2 * HB], b0_sb[:]
            )

            # products per row-group: R = (side_d + bd) * side_0, fp16
            # (walrus limits TensorScalarPtr out APs to 3 canonical dims)
            out_g0 = R3[:, UPB * ub : UPB * (ub + 1), 0:128]
            out_g1 = R3[:, UPB * ub : UPB * (ub + 1), 128:256]
            nc.vector.scalar_tensor_tensor(
                out_g0,
                ppd[:, 0:HB].rearrange("p (u v) -> p u v", v=128),
                bd_sb[:],
                s0[:, 0:HB].rearrange("p (u v) -> p u v", v=128),
                Alu.add,
                Alu.mult,
            )
            nc.vector.scalar_tensor_tensor(
                out_g1,
                ppd[:, HB : 2 * HB].rearrange("p (u v) -> p u v", v=128),
                bd_sb[:],
                s0[:, HB : 2 * HB].rearrange("p (u v) -> p u v", v=128),
                Alu.add,
                Alu.mult,
            )

def _phase2(tc, R, R3, cout):
    nc = tc.nc
    Copy = mybir.ActivationFunctionType.Copy
    # ---------------- Phase 2: C = sum_i A_i @ Bm_i^T ----------------
    with (
        tc.tile_pool(name="stg", bufs=2) as stg_pool,
        tc.tile_pool(name="pc", bufs=1, space="PSUM") as pc_pool,
        tc.tile_pool(name="co", bufs=1) as co_pool,
    ):
        pc0 = pc_pool.tile([128, 256], F32, tag="pc0")
        pc1 = pc_pool.tile([128, 256], F32, tag="pc1")
        pcs = [pc0, pc1]
        for ib in range(N // IB):
            stg = stg_pool.tile([64, IB * N], F16, tag="stg")
            nc.sync.dma_start(
                stg[:], R[64:128, ib * IB * N : (ib + 1) * IB * N]
            )
            stg3 = stg[:].rearrange("p (i v) -> p i v", v=N)
            for il in range(IB):
                i = ib * IB + il
                for nt in range(2):
                    nc.tensor.matmul(
                        pcs[nt][:],
                        R3[0:64, nt * 128 : (nt + 1) * 128, i : i + 1],
                        stg3[:, il, :],
                        start=(i == 0),
                        stop=(i == N - 1),
                        tile_position=(0, 0),
                    )
        c_sb = co_pool.tile([128, 512], F32)
        nc.scalar.activation(c_sb[:, 0:256], pc0[:], Copy, scale=INTERVAL)
        nc.scalar.activation(c_sb[:, 256:512], pc1[:], Copy, scale=INTERVAL)
        nc.sync.dma_start(cout[0:128, :], c_sb[:, 0:256])
        nc.sync.dma_start(cout[128:256, :], c_sb[:, 256:512])


def _build_nc():
    nc = bacc.Bacc("TRN2", target_bir_lowering=False)
    rbf0 = nc.dram_tensor("rbf0", [X, D], F32, kind="ExternalInput")
    rbfd = nc.dram_tensor("rbfd", [X, D], F32, kind="ExternalInput")
    w0 = nc.dram_tensor("w0", [128, 128], F16, kind="ExternalInput")
    wd = nc.dram_tensor("wd", [128, 128], F16, kind="ExternalInput")
    b0 = nc.dram_tensor("b0", [128, 1], F32, kind="ExternalInput")
    bd = nc.dram_tensor("bd", [128, 1], F32, kind="ExternalInput")
    cout = nc.dram_tensor("c", [N, N], F32, kind="ExternalOutput")
    with tile.TileContext(nc) as tc:
        with ExitStack() as ctx:
            _body(ctx, tc, rbf0, rbfd, w0, wd, b0, bd, cout)
    nc.compile()
    return nc


_CACHE = {}


def _get_nc():
    if "nc" not in _CACHE:
        _CACHE["nc"] = _build_nc()
    return _CACHE["nc"]


def _make_in_maps(inp):
    rbf_0 = np.ascontiguousarray(np.asarray(inp["rbf_0"], dtype=np.float32))
    rbf_d = np.ascontiguousarray(np.asarray(inp["rbf_d"], dtype=np.float32))

    # weight stacking: cols 0-63 = Q-family, 64-127 = R-family; the [64, 128]
    # block is duplicated across both partition halves for the two row-groups
    def wstack(wq, wr):
        wt = np.concatenate(
            [np.asarray(wq).T, np.asarray(wr).T], axis=1
        ).astype(np.float16)  # [64, 128]
        return np.concatenate([wt, wt], axis=0)  # [128, 128]

    def bstack(bq, br):
        return np.concatenate([np.asarray(bq), np.asarray(br)]).astype(
            np.float32
        )[:, None]  # [128, 1]

    w0 = wstack(inp["Q0_w"], inp["R0_w"])
    wd = wstack(inp["Q_w"], inp["R_w"])
    b0 = bstack(inp["Q0_b"], inp["R0_b"])
    bd = bstack(inp["Q_b"], inp["R_b"])

    return [
        {
            "rbf0": rbf_0[b].reshape(X, D),
            "rbfd": rbf_d[b].reshape(X, D),
            "w0": w0,
            "wd": wd,
            "b0": b0,
            "bd": bd,
        }
        for b in range(B)
    ]


def kernel(**inputs):
    in_maps = _make_in_maps(inputs)
    nc = _get_nc()
    res = run_bass_kernel_spmd(nc, in_maps, core_ids=list(range(B)))
    return np.stack([res.results[b]["c"] for b in range(B)], axis=0)


if __name__ == "__main__":
    import reference

    inp = {k: np.asarray(v) for k, v in reference.setup_inputs().items()}
    got = kernel(**inp)
    exp = np.asarray(reference.reference(**inp))
    err = np.abs(got - exp)
    print("absmax_err", err.max(), "rel", err.max() / np.abs(exp).max())

